# revision 1
# baseline (speedup 1.0000x reference)
"""Trainium2 Bass kernel for CombinedAdvancedLoss (focal + contrastive +
circularity + consensus), data-parallel over 8 NeuronCores.

Sharding: batch dim B=32 -> 4 items per core for logits/target/masks/
method_preds. features (1024x512) are passed to each core ROLLED by
-core*128 rows, so every core computes the same SPMD program on "its" 128
rows of the 1024x1024 similarity matrix (row sums / logsumexp are invariant
to the column permutation the roll induces; the diagonal lands in local
column block 0 and the positive pair in block 4).

Each core emits a [1,32] vector of linear partial sums; the host combines
them (the only nonlinear cross-core math - IoU ratios and the circularity
formula - acts on a handful of scalars).
"""

import sys

for _p in ("/opt/trn_rl_repo",):
    if _p not in sys.path:
        sys.path.insert(0, _p)

import numpy as np
import ml_dtypes

import concourse.bass as bass
import concourse.tile as tile
from concourse import mybir
from concourse.bass_utils import run_bass_kernel_spmd

import bass_rust as _bass_rust

# ---------------------------------------------------------------------------
# The walrus build in this container rejects >2 sync waits per instruction.
# Post-pass: hoist excess waits onto inserted same-engine NoOps.
_WAIT_CAP = 1


def _split_sync_waits(nc):
    n = 0
    for fn in nc.m.functions:
        for blk in fn.blocks:
            insts = blk.instructions
            i = 0
            while i < len(insts):
                inst = insts[i]
                si = inst.sync_info
                if si is not None and len(si.on_wait) > _WAIT_CAP:
                    waits = list(si.on_wait)
                    keep = waits[-_WAIT_CAP:]
                    extra = waits[:-_WAIT_CAP]
                    nops = []
                    for j in range(0, len(extra), _WAIT_CAP):
                        nop = mybir.InstDrain(
                            name=f"I-wsplit-{n}", engine=inst.engine)
                        n += 1
                        nop.sync_info = _bass_rust.SyncInfo(
                            on_wait=extra[j:j + _WAIT_CAP], on_update=[])
                        nops.append(nop)
                    inst.sync_info = _bass_rust.SyncInfo(
                        on_wait=keep, on_update=list(si.on_update))
                    for k, nop in enumerate(nops):
                        insts.insert(i + k, nop)
                    i += len(nops)
                i += 1
# ---------------------------------------------------------------------------

F32 = mybir.dt.float32
BF16 = mybir.dt.bfloat16
I32 = mybir.dt.int32
AF = mybir.ActivationFunctionType
OP = mybir.AluOpType
AX = mybir.AxisListType

NCORES = 8
B, C, H, W = 32, 8, 256, 256
BP = B // NCORES          # batch items per core (4)
HW = H * W                # 65536
FD = BP * HW // 128       # free dim of a full-core tile (2048)
XB = HW // 128            # free dim of one plane slice (512)
BF, DF = 1024, 512        # features shape
TEMP = 0.07
GAMMA_SCALE = 0.25        # ALPHA (0.25 for every class) * W_FOCAL
NPART = 32                # width of the per-core partials vector

# partials vector layout
K_FOCAL = 0               # sum 0.25*(1-p)^2 * ce
K_CONTRAST = 1            # sum (lse - pos) over this core's 128 rows
K_AREA = 2                # 4 cols: per-b mask area
K_EX = 6                  # 4 cols: per-b sum |dm/dh|
K_EY = 10                 # 8 cols: per-b (x2 chunks) sum |dm/dw|
K_S = 18                  # 3 cols: per-method sum of preds
K_I = 21                  # 3 cols: per-pair sum pi*pj  (01, 02, 12)
K_EXB = 26                # 4 cols: per-b boundary |m[128]-m[127]|


def _build_nc():
    nc = bass.Bass()

    lg = nc.declare_dram_parameter("lg", [BP, C, 128, XB], F32, isOutput=False)
    tg = nc.declare_dram_parameter("tg", [BP, 128, XB], I32, isOutput=False)
    mk = nc.declare_dram_parameter("mk", [BP, 2, 128, 256], F32, isOutput=False)
    mp = nc.declare_dram_parameter("mp", [3, BP, 128, XB], F32, isOutput=False)
    ft = nc.declare_dram_parameter("ft", [8, 128, DF], F32, isOutput=False)
    idf = nc.declare_dram_parameter("idf", [128, 128], F32, isOutput=False)
    idb = nc.declare_dram_parameter("idb", [128, 128], BF16, isOutput=False)
    zm = nc.declare_dram_parameter("zm", [128, 128], F32, isOutput=False)
    mb = nc.declare_dram_parameter("mb", [1, BP, 2, 256], F32, isOutput=False)
    out = nc.declare_dram_parameter("partials", [1, NPART], F32, isOutput=True)

    with tile.TileContext(nc) as tc:
        _emit(nc, tc, lg, tg, mk, mp, ft, idf, idb, zm, mb, out)
    _split_sync_waits(nc)
    return nc


def _emit(nc, tc, lg, tg, mk, mp, ft, idf, idb, zm, mb, out):
    from contextlib import ExitStack

    ctx = ExitStack()
    with ctx:
        singles = ctx.enter_context(tc.tile_pool(name="singles", bufs=1))
        lpool = ctx.enter_context(tc.tile_pool(name="lpool", bufs=3))
        qpool = ctx.enter_context(tc.tile_pool(name="qpool", bufs=3))
        mqpool = ctx.enter_context(tc.tile_pool(name="mqpool", bufs=3))
        spool = ctx.enter_context(tc.tile_pool(name="spool", bufs=2))
        ppool = ctx.enter_context(tc.tile_pool(name="ppool", bufs=1))
        fpool = ctx.enter_context(tc.tile_pool(name="fpool", bufs=1))
        scratch = ctx.enter_context(tc.tile_pool(name="scratch", bufs=1))
        tiny = ctx.enter_context(tc.tile_pool(name="tiny", bufs=1))
        cpool = ctx.enter_context(tc.tile_pool(name="cpool", bufs=2))
        pst = ctx.enter_context(tc.tile_pool(name="pst", bufs=2, space="PSUM"))
        pss = ctx.enter_context(tc.tile_pool(name="pss", bufs=1, space="PSUM"))
        psc = ctx.enter_context(tc.tile_pool(name="psc", bufs=2, space="PSUM"))
        psf = ctx.enter_context(tc.tile_pool(name="psf", bufs=1, space="PSUM"))

        # constants + accumulator
        ones = singles.tile([128, 1], F32)
        nc.vector.memset(ones, 1.0)
        acc = singles.tile([128, NPART], F32)
        nc.vector.memset(acc, 0.0)
        ident_f = singles.tile([128, 128], F32)
        nc.sync.dma_start(out=ident_f, in_=idf[:, :])
        ident_b = singles.tile([128, 128], BF16)
        nc.sync.dma_start(out=ident_b, in_=idb[:, :])
        zm_t = singles.tile([128, 128], F32)
        nc.sync.dma_start(out=zm_t, in_=zm[:, :])

        # ----------------- focal loss partials -----------------
        tg_t = singles.tile([128, BP, XB], I32)
        nc.sync.dma_start(out=tg_t, in_=tg.rearrange("b p x -> p b x"))
        tg_b = singles.tile([128, FD], BF16)
        nc.vector.tensor_copy(out=tg_b, in_=tg_t.rearrange("p b x -> p (b x)"))

        s_acc = None
        pt_acc = None
        q_prev = None
        mq_prev = None
        for c in range(C):
            l_c = lpool.tile([128, BP, XB], F32, tag="l")
            nc.sync.dma_start(out=l_c, in_=lg[:, c].rearrange("b p x -> p b x"))
            q_c = qpool.tile([128, FD], BF16, tag="q")
            nc.scalar.activation(
                out=q_c, in_=l_c.rearrange("p b x -> p (b x)"), func=AF.Exp
            )
            mq_c = mqpool.tile([128, FD], BF16, tag="mq")
            nc.vector.scalar_tensor_tensor(
                out=mq_c, in0=tg_b, scalar=float(c), in1=q_c,
                op0=OP.is_equal, op1=OP.mult,
            )
            if c == 0:
                q_prev, mq_prev = q_c, mq_c
            elif c == 1:
                s_acc = spool.tile([128, FD], BF16, tag="s")
                nc.vector.tensor_tensor(out=s_acc, in0=q_prev, in1=q_c, op=OP.add)
                pt_acc = spool.tile([128, FD], BF16, tag="pt")
                nc.vector.tensor_tensor(out=pt_acc, in0=mq_prev, in1=mq_c, op=OP.add)
                q_prev = mq_prev = None
            else:
                s_new = spool.tile([128, FD], BF16, tag="s")
                nc.vector.tensor_tensor(out=s_new, in0=s_acc, in1=q_c, op=OP.add)
                s_acc = s_new
                pt_new = spool.tile([128, FD], BF16, tag="pt")
                nc.vector.tensor_tensor(out=pt_new, in0=pt_acc, in1=mq_c, op=OP.add)
                pt_acc = pt_new

        ln_s = scratch.tile([128, FD], BF16, tag="lns")
        nc.scalar.activation(out=ln_s, in_=s_acc, func=AF.Ln)
        ln_pt = scratch.tile([128, FD], BF16, tag="lnpt")
        nc.scalar.activation(out=ln_pt, in_=pt_acc, func=AF.Ln)
        ce = scratch.tile([128, FD], BF16, tag="ce")
        nc.vector.tensor_tensor(out=ce, in0=ln_s, in1=ln_pt, op=OP.subtract)
        p_t = scratch.tile([128, FD], BF16, tag="p")
        nc.scalar.activation(out=p_t, in_=ce, func=AF.Exp, scale=-1.0)
        u_t = scratch.tile([128, FD], BF16, tag="u")
        nc.vector.tensor_scalar(
            out=u_t, in0=p_t, scalar1=-1.0, scalar2=1.0, op0=OP.mult, op1=OP.add
        )
        v_t = scratch.tile([128, FD], BF16, tag="v")
        nc.vector.tensor_tensor(out=v_t, in0=u_t, in1=u_t, op=OP.mult)
        w_t = scratch.tile([128, FD], BF16, tag="wt")
        nc.vector.tensor_tensor(out=w_t, in0=v_t, in1=ce, op=OP.mult)
        w_junk = scratch.tile([128, FD], BF16, tag="wj")
        nc.vector.tensor_scalar(
            out=w_junk, in0=w_t, scalar1=GAMMA_SCALE, scalar2=0.0,
            op0=OP.mult, op1=OP.add, accum_out=acc[:, K_FOCAL:K_FOCAL + 1],
        )

        # ----------------- consensus partials -----------------
        p_tiles = []
        for i in range(3):
            p_i = ppool.tile([128, BP, XB], F32, tag=f"mp{i}")
            nc.sync.dma_start(out=p_i, in_=mp[i].rearrange("b p x -> p b x"))
            p_tiles.append(p_i)
            sj = scratch.tile([128, FD], BF16, tag="wj")
            nc.vector.tensor_scalar(
                out=sj, in0=p_i.rearrange("p b x -> p (b x)"), scalar1=1.0,
                scalar2=0.0, op0=OP.mult, op1=OP.add,
                accum_out=acc[:, K_S + i:K_S + i + 1],
            )
        for k, (i, j) in enumerate(((0, 1), (0, 2), (1, 2))):
            ij = scratch.tile([128, FD], BF16, tag="wt")
            nc.vector.tensor_tensor(
                out=ij, in0=p_tiles[i].rearrange("p b x -> p (b x)"),
                in1=p_tiles[j].rearrange("p b x -> p (b x)"), op=OP.mult,
            )
            ij2 = scratch.tile([128, FD], BF16, tag="wj")
            nc.vector.tensor_scalar(
                out=ij2, in0=ij, scalar1=1.0, scalar2=0.0,
                op0=OP.mult, op1=OP.add,
                accum_out=acc[:, K_I + k:K_I + k + 1],
            )

        # ----------------- circularity partials -----------------
        m_t = singles.tile([128, BP, 2, 256], F32)
        nc.sync.dma_start(out=m_t, in_=mk.rearrange("b c p w -> p b c w"))
        for b in range(BP):
            ps_b = psc.tile([128, 2, 256], F32, tag="circ")
            nc.tensor.matmul(
                out=ps_b, lhsT=zm_t, rhs=m_t[:, b], start=True, stop=True
            )
            nc.vector.tensor_reduce(
                out=acc[:, K_EX + b:K_EX + b + 1], in_=ps_b,
                axis=AX.XY, op=OP.add, apply_absolute_value=True,
            )
            aj = scratch.tile([128, XB], BF16, tag="actj")
            nc.scalar.activation(
                out=aj, in_=m_t[:, b].rearrange("p c w -> p (c w)"), func=AF.Copy,
                accum_out=acc[:, K_AREA + b:K_AREA + b + 1],
            )
        mb_t = singles.tile([1, BP, 2, 256], F32)
        nc.sync.dma_start(out=mb_t, in_=mb[:, :, :, :])
        d_bnd = cpool.tile([1, BP, 256], BF16, tag="dbnd")
        nc.vector.tensor_tensor(
            out=d_bnd, in0=mb_t[:, :, 1], in1=mb_t[:, :, 0], op=OP.subtract
        )
        nc.vector.tensor_reduce(
            out=acc[0:1, K_EXB:K_EXB + BP], in_=d_bnd,
            axis=AX.X, op=OP.add, apply_absolute_value=True,
        )
        d_y = singles.tile([128, BP, 2, 255], BF16)
        nc.vector.tensor_tensor(
            out=d_y, in0=m_t[:, :, :, 1:256], in1=m_t[:, :, :, 0:255],
            op=OP.subtract,
        )
        nc.vector.tensor_reduce(
            out=acc[:, K_EY:K_EY + 8].rearrange("p (b c) -> p b c", b=BP),
            in_=d_y, axis=AX.X, op=OP.add, apply_absolute_value=True,
        )

        # ----------------- contrastive partials -----------------
        f_t = fpool.tile([128, 8, DF], F32)
        nc.sync.dma_start(out=f_t, in_=ft.rearrange("k p d -> p k d"))
        ss = tiny.tile([128, 8], F32, tag="ss")
        for k in range(8):
            fsq = scratch.tile([128, DF], BF16, tag="actj")
            nc.scalar.activation(
                out=fsq, in_=f_t[:, k], func=AF.Square,
                accum_out=ss[:, k:k + 1],
            )
        # rsqrt via exp(-0.5*ln(ss)) (stays in the exp/ln table set),
        # then one Newton step y' = y*(1.5 - 0.5*ss*y^2)
        lns_t = tiny.tile([128, 8], F32, tag="lnss")
        nc.scalar.activation(out=lns_t, in_=ss, func=AF.Ln)
        inv0 = tiny.tile([128, 8], F32, tag="inv0")
        nc.scalar.activation(out=inv0, in_=lns_t, func=AF.Exp, scale=-0.5)
        t1 = tiny.tile([128, 8], F32, tag="t1")
        nc.vector.tensor_tensor(out=t1, in0=inv0, in1=inv0, op=OP.mult)
        t2 = tiny.tile([128, 8], F32, tag="t2")
        nc.vector.tensor_tensor(out=t2, in0=t1, in1=ss, op=OP.mult)
        t3 = tiny.tile([128, 8], F32, tag="t3")
        nc.vector.tensor_scalar(
            out=t3, in0=t2, scalar1=-0.5, scalar2=1.5, op0=OP.mult, op1=OP.add
        )
        inv = tiny.tile([128, 8], F32, tag="inv")
        nc.vector.tensor_tensor(out=inv, in0=inv0, in1=t3, op=OP.mult)

        fn = fpool.tile([128, 8, DF], BF16)
        for k in range(8):
            nc.vector.tensor_scalar(
                out=fn[:, k], in0=f_t[:, k], scalar1=inv[:, k:k + 1],
                scalar2=None, op0=OP.mult,
            )
        ftr = [
            fpool.tile([128, 8, 128], BF16, tag=f"ftr{dc}", name=f"ftr{dc}")
            for dc in range(4)
        ]
        for k in range(8):
            for dc in range(4):
                ps_t = pst.tile([128, 128], BF16, tag="tr")
                nc.tensor.transpose(
                    out=ps_t, in_=fn[:, k, dc * 128:(dc + 1) * 128],
                    identity=ident_b,
                )
                nc.vector.tensor_copy(out=ftr[dc][:, k], in_=ps_t)
        sim = []
        for half in range(2):
            ps_h = pss.tile([128, 512], F32, tag=f"sim{half}")
            for dc in range(4):
                nc.tensor.matmul(
                    out=ps_h,
                    lhsT=ftr[dc][:, 0],
                    rhs=ftr[dc].rearrange("p k x -> p (k x)")[
                        :, half * 512:(half + 1) * 512],
                    start=(dc == 0), stop=(dc == 3),
                )
            sim.append(ps_h)
        # rolled features: diagonal = local column block 0, positive = block 4
        nc.vector.scalar_tensor_tensor(
            out=sim[0][:, 0:128], in0=ident_f, scalar=-1e4,
            in1=sim[0][:, 0:128], op0=OP.mult, op1=OP.add,
        )
        rsum = tiny.tile([128, 2], F32, tag="rsum")
        for half in range(2):
            e_h = scratch.tile([128, 512], BF16, tag="actj")
            nc.scalar.activation(
                out=e_h, in_=sim[half], func=AF.Exp, scale=1.0 / TEMP,
                accum_out=rsum[:, half:half + 1],
            )
        rtot = tiny.tile([128, 1], F32, tag="rtot")
        nc.vector.tensor_tensor(
            out=rtot, in0=rsum[:, 0:1], in1=rsum[:, 1:2], op=OP.add
        )
        lse = tiny.tile([128, 1], F32, tag="lse")
        nc.scalar.activation(out=lse, in_=rtot, func=AF.Ln)
        posj = scratch.tile([128, 128], F32, tag="posj")
        pos = tiny.tile([128, 1], F32, tag="pos")
        nc.vector.tensor_tensor(
            out=posj, in0=sim[1][:, 0:128], in1=ident_f, op=OP.mult
        )
        posj2 = scratch.tile([128, 128], BF16, tag="posj2")
        nc.vector.tensor_scalar(
            out=posj2, in0=posj, scalar1=1.0 / TEMP, scalar2=0.0,
            op0=OP.mult, op1=OP.add, accum_out=pos,
        )
        nc.vector.tensor_tensor(
            out=acc[:, K_CONTRAST:K_CONTRAST + 1], in0=lse, in1=pos,
            op=OP.subtract,
        )

        # ----------------- partition-reduce + store -----------------
        pfin = psf.tile([1, NPART], F32)
        nc.tensor.matmul(out=pfin, lhsT=ones, rhs=acc, start=True, stop=True)
        out_t = tiny.tile([1, NPART], F32, tag="outt")
        nc.vector.tensor_copy(out=out_t, in_=pfin)
        nc.sync.dma_start(out=out[:, :], in_=out_t)


def _zmat():
    ident = np.eye(128, dtype=np.float32)
    z = np.roll(ident, -1, axis=0) - ident
    z[:, 127] = 0.0
    return np.ascontiguousarray(z)


def _host_inputs(logits, target, features, masks, method_preds):
    """Slice/reshape full inputs into per-core input maps."""
    ident = np.eye(128, dtype=np.float32)
    consts = {
        "idf": ident,
        "idb": ident.astype(ml_dtypes.bfloat16),
        # zm = P127 @ (Cyc - I): row-diff matrix with output row 127 zeroed
        "zm": _zmat(),
    }
    in_maps = []
    for c in range(NCORES):
        b0 = c * BP
        in_maps.append({
            "lg": np.ascontiguousarray(
                logits[b0:b0 + BP].reshape(BP, C, 128, XB)),
            "tg": np.ascontiguousarray(
                target[b0:b0 + BP].reshape(BP, 128, XB)),
            "mk": np.ascontiguousarray(
                masks[b0:b0 + BP, 0].reshape(BP, 2, 128, 256)),
            "mp": np.ascontiguousarray(
                method_preds[:, b0:b0 + BP].reshape(3, BP, 128, XB)),
            "mb": np.ascontiguousarray(
                masks[b0:b0 + BP, 0, 127:129, :].reshape(1, BP, 2, 256)),
            "ft": np.ascontiguousarray(
                np.roll(features, -c * 128, axis=0).reshape(8, 128, DF)),
            **consts,
        })
    return in_maps


def _combine(partials):
    """Host-side combination of the per-core [1,32] partial vectors."""
    P = np.stack([np.asarray(p).reshape(-1).astype(np.float64)
                  for p in partials])  # [8,32]
    focal = P[:, K_FOCAL].sum() / (B * HW)
    contrast = 0.5 * P[:, K_CONTRAST].sum() / BF

    circ_total = 0.0
    for c in range(NCORES):
        for b in range(BP):
            area = P[c, K_AREA + b]
            ex = P[c, K_EX + b] + P[c, K_EXB + b]
            ey = P[c, K_EY + 2 * b] + P[c, K_EY + 2 * b + 1]
            per = ex + ey
            if area > 0 and per > 0:
                circv = 4.0 * np.pi * area / max(per, 1e-12) ** 2
                circ_total += (circv - 1.0) ** 2
    circ = 0.1 * circ_total / B

    S = P[:, K_S:K_S + 3].sum(axis=0)
    I = P[:, K_I:K_I + 3].sum(axis=0)
    cons_total = 0.0
    for k, (i, j) in enumerate(((0, 1), (0, 2), (1, 2))):
        union = S[i] + S[j] - I[k]
        iou = I[k] / (union + 1e-6)
        cons_total += max(0.6 - iou, 0.0)
    consensus = 0.3 * cons_total / 3.0

    return np.float32(focal + contrast + circ + consensus)


_CACHED_NC = None


def _get_nc():
    global _CACHED_NC
    if _CACHED_NC is None:
        _CACHED_NC = _build_nc()
    return _CACHED_NC


def kernel(logits, target, features, masks, method_preds):
    logits = np.asarray(logits, dtype=np.float32)
    target = np.asarray(target, dtype=np.int32)
    features = np.asarray(features, dtype=np.float32)
    masks = np.asarray(masks, dtype=np.float32)
    method_preds = np.asarray(method_preds, dtype=np.float32)

    in_maps = _host_inputs(logits, target, features, masks, method_preds)
    res = run_bass_kernel_spmd(_get_nc(), in_maps, list(range(NCORES)))
    partials = [res.results[c]["partials"] for c in range(NCORES)]
    return _combine(partials)



# revision 9
# speedup vs baseline: 1.6968x; 1.6968x over previous
"""Trainium2 Bass kernel for CombinedAdvancedLoss (focal + contrastive +
circularity + consensus), data-parallel over 8 NeuronCores.

Sharding: batch dim B=32 -> 4 items per core for logits/target/masks/
method_preds. features (1024x512) are passed to each core TRANSPOSED and
ROLLED by -core*128 rows, so every core computes the same SPMD program on
"its" 128 rows of the 1024x1024 similarity matrix (the diagonal lands in
local column block 0 and the positive pair in block 4).

Engine split per core (all inputs pre-cast to bf16 on host):
  ScalarE: exp(logits), ln(S), p=exp(-ce), |row-diff| accum, norm chain
  VectorE: one-hot masks + masked-select chain, pair products, col-diffs
  TensorE: S=sum_c exp via identity-matmul PSUM accumulation; large
           free-axis sums via ones-matmuls routed into a PSUM accumulator;
           feature Gram matrix; row-diff stencil matmuls
Each core emits two small partial vectors; the host combines them (the only
nonlinear cross-core math - IoU ratios, circularity - acts on a few scalars).
"""

import sys

for _p in ("/opt/trn_rl_repo",):
    if _p not in sys.path:
        sys.path.insert(0, _p)

import numpy as np
import ml_dtypes

import concourse.bass as bass
import concourse.tile as tile
from concourse import mybir
from concourse.bass_utils import run_bass_kernel_spmd

import bass_rust as _bass_rust

# ---------------------------------------------------------------------------
# The walrus build in this container rejects >2 sync waits per instruction.
# Post-pass: hoist excess waits onto inserted same-engine NoOps.
_WAIT_CAP = 1


def _split_sync_waits(nc):
    n = 0
    for fn in nc.m.functions:
        for blk in fn.blocks:
            insts = blk.instructions
            i = 0
            while i < len(insts):
                inst = insts[i]
                si = inst.sync_info
                if si is not None and len(si.on_wait) > _WAIT_CAP:
                    waits = list(si.on_wait)
                    keep = waits[-_WAIT_CAP:]
                    extra = waits[:-_WAIT_CAP]
                    nops = []
                    for j in range(0, len(extra), _WAIT_CAP):
                        nop = mybir.InstNoOp(
                            name=f"I-wsplit-{n}", engine=inst.engine)
                        n += 1
                        nop.sync_info = _bass_rust.SyncInfo(
                            on_wait=extra[j:j + _WAIT_CAP], on_update=[])
                        nops.append(nop)
                    inst.sync_info = _bass_rust.SyncInfo(
                        on_wait=keep, on_update=list(si.on_update))
                    for k, nop in enumerate(nops):
                        insts.insert(i + k, nop)
                    i += len(nops)
                i += 1
# ---------------------------------------------------------------------------

F32 = mybir.dt.float32
BF16 = mybir.dt.bfloat16
AF = mybir.ActivationFunctionType
OP = mybir.AluOpType
AX = mybir.AxisListType

NCORES = 8
B, C, H, W = 32, 8, 256, 256
BP = B // NCORES          # batch items per core (4)
HW = H * W                # 65536
FD = BP * HW // 128       # free dim of a full-core tile (2048)
BF, DF = 1024, 512        # features shape
TEMP = 0.07
GAMMA_SCALE = 0.25        # ALPHA (0.25 for every class) * W_FOCAL

# SBUF acc [128, NACC] columns (partition-wise partials; summed via ones-MM)
NACC = 32
K_FOCAL = 0               # TTR: sum 0.25*(1-p)^2 * ce
K_CONTRAST = 1            # per-row lse - pos
K_EX = 2                  # 4 cols: per-b |row-diff| (within chunks)
K_EY = 6                  # 8 cols: per-(b,chunk) |col-diff|
K_BND = 14                # 4 cols: per-b boundary |m[128]-m[127]| (part 0)

# PSUM accP [32, 1024] rows (free-axis sums via routed ones-MMs)
R_AREA = 0                # 4 rows: per-b mask area
R_S = 4                   # 3 rows: per-method sum of preds
R_I = 7                   # 3 rows: per-pair sum pi*pj (01, 02, 12)
R_W = 10                  # 1 row: sum (1-p)^2 * ce (x0.25 on host)
NROW = 32


def _build_nc():
    nc = bass.Bass()

    lg = nc.declare_dram_parameter("lg", [C, 128, FD], BF16, isOutput=False)
    tg = nc.declare_dram_parameter("tg", [128, FD], BF16, isOutput=False)
    mk = nc.declare_dram_parameter("mk", [128, BP, 2, 256], BF16, isOutput=False)
    mb = nc.declare_dram_parameter("mb", [1, BP, 2, 256], BF16, isOutput=False)
    mp = nc.declare_dram_parameter("mp", [3, 128, FD], BF16, isOutput=False)
    ft = nc.declare_dram_parameter("ft", [128, 4, BF], BF16, isOutput=False)
    idb = nc.declare_dram_parameter("idb", [128, 128], BF16, isOutput=False)
    zmb = nc.declare_dram_parameter("zmb", [128, 128], BF16, isOutput=False)
    ohb = nc.declare_dram_parameter("ohb", [128, 63], BF16, isOutput=False)
    onesr = nc.declare_dram_parameter("onesr", [1, 128], BF16, isOutput=False)
    pa = nc.declare_dram_parameter("pa", [1, NACC], F32, isOutput=True)
    pb = nc.declare_dram_parameter("pb", [NROW, 1], F32, isOutput=True)

    with tile.TileContext(nc) as tc:
        _emit(nc, tc, lg, tg, mk, mb, mp, ft, idb, zmb, ohb, onesr, pa, pb)
    _split_sync_waits(nc)
    return nc


def _emit(nc, tc, lg, tg, mk, mb, mp, ft, idb, zmb, ohb, onesr, pa, pb):
    from contextlib import ExitStack

    ctx = ExitStack()
    with ctx:
        singles = ctx.enter_context(tc.tile_pool(name="singles", bufs=1))
        lpool = ctx.enter_context(tc.tile_pool(name="lpool", bufs=3))
        qpool = ctx.enter_context(tc.tile_pool(name="qpool", bufs=2))
        mpool = ctx.enter_context(tc.tile_pool(name="mpool", bufs=2))
        selpool = ctx.enter_context(tc.tile_pool(name="selpool", bufs=2))
        lselpool = ctx.enter_context(tc.tile_pool(name="lselpool", bufs=2))
        sqpool = ctx.enter_context(tc.tile_pool(name="sqpool", bufs=2))
        prodpool = ctx.enter_context(tc.tile_pool(name="prodpool", bufs=2))
        scratch = ctx.enter_context(tc.tile_pool(name="scratch", bufs=1))
        tiny = ctx.enter_context(tc.tile_pool(name="tiny", bufs=1))
        pAcc = ctx.enter_context(
            tc.tile_pool(name="pAcc", bufs=1, space="PSUM"))

        # ---------- constants ----------
        id_t = singles.tile([128, 128], BF16)
        nc.sync.dma_start(out=id_t, in_=idb[:, :])
        tg_t = singles.tile([128, FD], BF16)
        nc.sync.dma_start(out=tg_t, in_=tg[:, :])
        oh_t = singles.tile([128, 63], BF16)
        nc.sync.dma_start(out=oh_t, in_=ohb[:, :])
        zm_t = singles.tile([128, 128], BF16)
        nc.sync.dma_start(out=zm_t, in_=zmb[:, :])
        ones_r = singles.tile([1, 128], BF16)
        nc.sync.dma_start(out=ones_r, in_=onesr[:, :])

        acc = singles.tile([128, NACC], F32)
        nc.vector.memset(acc, 0.0)
        onesf = singles.tile([128, 1], F32)
        nc.vector.memset(onesf, 1.0)

        accP = pAcc.tile([NROW, 512], F32)

        # ---------- focal: channel loop (S on PE, l_sel on DVE) ----------
        l_tiles = []
        lsel = None
        with tc.tile_pool(name="pS", bufs=1, space="PSUM") as pS, \
             tc.tile_pool(name="pSS", bufs=1, space="PSUM") as pSS:
            sP = [pS.tile([128, 512], F32, tag=f"s{h}", name=f"s{h}")
                  for h in range(4)]
            ssP = pSS.tile([1, BF], F32)

            for c in range(C):
                l_c = lpool.tile([128, FD], BF16, tag="l")
                nc.sync.dma_start(out=l_c, in_=lg[c])
                q_c = qpool.tile([128, FD], BF16, tag="q")
                nc.scalar.activation(out=q_c, in_=l_c, func=AF.Exp)
                for h in range(4):
                    nc.tensor.matmul(
                        out=sP[h], lhsT=id_t,
                        rhs=q_c[:, h * 512:(h + 1) * 512],
                        start=(c == 0), stop=(c == C - 1),
                    )
                m_c = mpool.tile([128, FD], BF16, tag="m")
                nc.vector.tensor_scalar(
                    out=m_c, in0=tg_t, scalar1=float(c), scalar2=None,
                    op0=OP.is_equal,
                )
                if c == 0:
                    lsel = lselpool.tile([128, FD], BF16, tag="lsel")
                    nc.vector.tensor_tensor(
                        out=lsel, in0=m_c, in1=l_c, op=OP.mult)
                else:
                    ms = selpool.tile([128, FD], BF16, tag="msel")
                    nc.vector.tensor_tensor(
                        out=ms, in0=m_c, in1=l_c, op=OP.mult)
                    ls_new = lselpool.tile([128, FD], BF16, tag="lsel")
                    nc.vector.tensor_tensor(
                        out=ls_new, in0=lsel, in1=ms, op=OP.add)
                    lsel = ls_new

            # lnS (frees the S banks on scope exit)
            lns = scratch.tile([128, FD], BF16, tag="lns")
            for h in range(4):
                nc.scalar.activation(
                    out=lns[:, h * 512:(h + 1) * 512], in_=sP[h], func=AF.Ln)

            # contrastive norms: ss = sum_d ft^2 per global row (PE ones-MM)
            mk_t = singles.tile([128, BP, 2, 256], BF16)
            nc.sync.dma_start(out=mk_t, in_=mk[:, :, :, :])
            mb_t = singles.tile([1, BP, 2, 256], BF16)
            nc.sync.dma_start(out=mb_t, in_=mb[:, :, :, :])
            mp_t = [
                singles.tile([128, FD], BF16, name=f"mp{i}") for i in range(3)
            ]
            for i in range(3):
                nc.sync.dma_start(out=mp_t[i], in_=mp[i])
            ft_t = singles.tile([128, 4, BF], BF16)
            nc.sync.dma_start(out=ft_t, in_=ft[:, :, :])

            for dc in range(4):
                sq = sqpool.tile([128, BF], BF16, tag="sq")
                nc.vector.tensor_tensor(
                    out=sq, in0=ft_t[:, dc], in1=ft_t[:, dc], op=OP.mult)
                for h in range(2):
                    nc.tensor.matmul(
                        out=ssP[:, h * 512:(h + 1) * 512],
                        lhsT=oh_t[:, 31:32],
                        rhs=sq[:, h * 512:(h + 1) * 512],
                        start=(dc == 0), stop=(dc == 3),
                        skip_group_check=True,
                    )
            # rn = 1/sqrt(ss) via exp(-0.5*ln(ss)) (stays in exp/ln table set)
            lnss = tiny.tile([1, BF], F32, tag="lnss")
            nc.scalar.activation(out=lnss, in_=ssP, func=AF.Ln)
            rnrow = tiny.tile([1, BF], BF16, tag="rnrow")
            nc.scalar.activation(out=rnrow, in_=lnss, func=AF.Exp, scale=-0.5)

        # ---------- consensus sums (PE ones-MMs into accP rows) ----------
        first = [True]

        def accmm(q, rhs, stop=False):
            n = rhs.free_size()
            chunks = [(h, min(512, n - h)) for h in range(0, n, 512)]
            for ci, (h, w) in enumerate(chunks):
                nc.tensor.matmul(
                    out=accP[:, 0:w],
                    lhsT=oh_t[:, 31 - q:63 - q],
                    rhs=rhs[:, h:h + w],
                    start=first[0],
                    stop=stop and ci == len(chunks) - 1,
                    skip_group_check=True,
                )
                first[0] = False

        # first accP matmul is full-width N=1024 so every accumulator
        # element gets its has_written bit set before narrower adds.
        for i in range(3):
            accmm(R_S + i, mp_t[i])
        for b in range(BP):
            accmm(R_AREA + b, mk_t[:, b].rearrange("p c w -> p (c w)"))
        for k, (i, j) in enumerate(((0, 1), (0, 2), (1, 2))):
            pr = prodpool.tile([128, FD], BF16, tag="prod")
            nc.vector.tensor_tensor(
                out=pr, in0=mp_t[i], in1=mp_t[j], op=OP.mult)
            accmm(R_I + k, pr)

        # ---------- focal tail (DVE/ACT) ----------
        ce = scratch.tile([128, FD], BF16, tag="ce")
        nc.vector.tensor_tensor(out=ce, in0=lns, in1=lsel, op=OP.subtract)
        p_t = scratch.tile([128, FD], BF16, tag="p")
        nc.scalar.activation(out=p_t, in_=ce, func=AF.Exp, scale=-1.0)
        u_t = scratch.tile([128, FD], BF16, tag="u")
        nc.vector.tensor_scalar(
            out=u_t, in0=p_t, scalar1=-1.0, scalar2=1.0,
            op0=OP.mult, op1=OP.add,
        )
        v_t = scratch.tile([128, FD], BF16, tag="v")
        nc.vector.tensor_tensor(out=v_t, in0=u_t, in1=u_t, op=OP.mult)
        w_t = scratch.tile([128, FD], BF16, tag="w")
        nc.vector.tensor_tensor(out=w_t, in0=v_t, in1=ce, op=OP.mult)
        accmm(R_W, w_t, stop=True)

        # ---------- circularity: col-diffs + boundary (DVE) ----------
        d_y = scratch.tile([128, BP, 2, 255], BF16, tag="dy")
        nc.vector.tensor_tensor(
            out=d_y, in0=mk_t[:, :, :, 1:256], in1=mk_t[:, :, :, 0:255],
            op=OP.subtract,
        )
        nc.vector.tensor_reduce(
            out=acc[:, K_EY:K_EY + 8].rearrange("p (b c) -> p b c", b=BP),
            in_=d_y, axis=AX.X, op=OP.add, apply_absolute_value=True,
        )
        d_b = tiny.tile([1, BP, 256], BF16, tag="db")
        nc.vector.tensor_tensor(
            out=d_b, in0=mb_t[:, :, 1], in1=mb_t[:, :, 0], op=OP.subtract)
        nc.vector.tensor_reduce(
            out=acc[0:1, K_BND:K_BND + BP], in_=d_b,
            axis=AX.X, op=OP.add, apply_absolute_value=True,
        )

        # ---------- contrastive: Gram + lse (PE/DVE/ACT) ----------
        gs = scratch.tile([128, BF], BF16, tag="gs")
        rnb_sb = scratch.tile([128, BF], BF16, tag="rnb")
        with tc.tile_pool(name="pG", bufs=1, space="PSUM") as pG, \
             tc.tile_pool(name="pR", bufs=1, space="PSUM") as pR:
            gP = pG.tile([128, BF], F32)
            for dc in range(4):
                for h in range(2):
                    nc.tensor.matmul(
                        out=gP[:, h * 512:(h + 1) * 512],
                        lhsT=ft_t[:, dc, 0:128],
                        rhs=ft_t[:, dc, h * 512:(h + 1) * 512],
                        start=(dc == 0), stop=(dc == 3),
                        skip_group_check=True,
                    )
            rnbP = pR.tile([128, BF], F32)
            for h in range(2):
                nc.tensor.matmul(
                    out=rnbP[:, h * 512:(h + 1) * 512], lhsT=ones_r,
                    rhs=rnrow[:, h * 512:(h + 1) * 512],
                    start=True, stop=True, skip_group_check=True)
            nc.scalar.activation(out=rnb_sb, in_=rnbP, func=AF.Copy)
            nc.vector.tensor_tensor(out=gs, in0=gP, in1=rnb_sb, op=OP.mult)

        # rn for this core's rows = diag of rnb block 0
        rdj = scratch.tile([128, 128], BF16, tag="rdj")
        nc.vector.tensor_tensor(
            out=rdj, in0=rnb_sb[:, 0:128], in1=id_t, op=OP.mult)
        rn_r = tiny.tile([128, 1], F32, tag="rnr")
        nc.vector.tensor_reduce(
            out=rn_r, in_=rdj, axis=AX.X, op=OP.add)
        gq = scratch.tile([128, BF], BF16, tag="gq")
        nc.vector.tensor_scalar(
            out=gq, in0=gs, scalar1=rn_r, scalar2=None, op0=OP.mult)
        # positive pair: diag of column block 4 (before diag masking)
        pj = scratch.tile([128, 128], BF16, tag="pj")
        nc.vector.tensor_tensor(
            out=pj, in0=gq[:, 512:640], in1=id_t, op=OP.mult)
        posu = tiny.tile([128, 1], F32, tag="posu")
        nc.vector.tensor_reduce(out=posu, in_=pj, axis=AX.X, op=OP.add)
        # mask own-diagonal (column block 0) to -inf-ish
        nc.vector.scalar_tensor_tensor(
            out=gq[:, 0:128], in0=id_t, scalar=-1e4, in1=gq[:, 0:128],
            op0=OP.mult, op1=OP.add,
        )
        esum = tiny.tile([128, 1], F32, tag="esum")
        junkB = scratch.tile([128, BF], BF16, tag="junkB")
        nc.scalar.activation(
            out=junkB, in_=gq, func=AF.Exp, scale=1.0 / TEMP, accum_out=esum)
        lse = tiny.tile([128, 1], F32, tag="lse")
        nc.scalar.activation(out=lse, in_=esum, func=AF.Ln)
        nc.vector.scalar_tensor_tensor(
            out=acc[:, K_CONTRAST:K_CONTRAST + 1], in0=posu,
            scalar=-1.0 / TEMP, in1=lse, op0=OP.mult, op1=OP.add,
        )

        # ---------- circularity: row-diffs via stencil matmul ----------
        junkC = scratch.tile([128, 512], BF16, tag="junkC")
        with tc.tile_pool(name="pZ", bufs=2, space="PSUM") as pZ, \
             tc.tile_pool(name="pF", bufs=1, space="PSUM") as pF:
            for b in range(BP):
                zP = pZ.tile([128, 512], F32, tag="z")
                nc.tensor.matmul(
                    out=zP, lhsT=zm_t,
                    rhs=mk_t[:, b].rearrange("p c w -> p (c w)"),
                    start=True, stop=True,
                )
                nc.scalar.activation(
                    out=junkC, in_=zP, func=AF.Abs,
                    accum_out=acc[:, K_EX + b:K_EX + b + 1],
                )

            # ---------- finalize ----------
            junkD = scratch.tile([NROW, 512], BF16, tag="junkD")
            pb_sb = tiny.tile([NROW, 1], F32, tag="pbs")
            nc.scalar.activation(
                out=junkD, in_=accP, func=AF.Copy, accum_out=pb_sb)
            nc.sync.dma_start(out=pb[:, :], in_=pb_sb)

            pfin = pF.tile([1, NACC], F32)
            nc.tensor.matmul(
                out=pfin, lhsT=onesf, rhs=acc, start=True, stop=True)
            pa_sb = tiny.tile([1, NACC], F32, tag="pas")
            nc.vector.tensor_copy(out=pa_sb, in_=pfin)
            nc.sync.dma_start(out=pa[:, :], in_=pa_sb)


def _zmat():
    ident = np.eye(128, dtype=np.float32)
    z = np.roll(ident, -1, axis=0) - ident
    z[:, 127] = 0.0
    return np.ascontiguousarray(z)


def _host_inputs(logits, target, features, masks, method_preds):
    """Slice/reshape/cast full inputs into per-core input maps (bf16)."""
    bf = ml_dtypes.bfloat16
    ident = np.eye(128, dtype=np.float32)
    ohb = np.zeros((128, 63), dtype=np.float32)
    ohb[:, 31] = 1.0
    consts = {
        "idb": ident.astype(bf),
        "zmb": _zmat().astype(bf),
        "ohb": ohb.astype(bf),
        "onesr": np.ones((1, 128), dtype=np.float32).astype(bf),
    }
    in_maps = []
    for c in range(NCORES):
        b0 = c * BP
        lgc = (logits[b0:b0 + BP].reshape(BP, C, 128, 512)
               .transpose(1, 2, 0, 3).reshape(C, 128, FD))
        tgc = (target[b0:b0 + BP].reshape(BP, 128, 512)
               .transpose(1, 0, 2).reshape(128, FD))
        mkc = (masks[b0:b0 + BP, 0].reshape(BP, 2, 128, 256)
               .transpose(2, 0, 1, 3))
        mbc = masks[b0:b0 + BP, 0, 127:129, :].reshape(1, BP, 2, 256)
        mpc = (method_preds[:, b0:b0 + BP].reshape(3, BP, 128, 512)
               .transpose(0, 2, 1, 3).reshape(3, 128, FD))
        ftc = (np.roll(features, -c * 128, axis=0).T
               .reshape(4, 128, BF).transpose(1, 0, 2))
        in_maps.append({
            "lg": np.ascontiguousarray(lgc).astype(bf),
            "tg": np.ascontiguousarray(tgc.astype(np.float32)).astype(bf),
            "mk": np.ascontiguousarray(mkc).astype(bf),
            "mb": np.ascontiguousarray(mbc).astype(bf),
            "mp": np.ascontiguousarray(mpc).astype(bf),
            "ft": np.ascontiguousarray(ftc).astype(bf),
            **consts,
        })
    return in_maps


def _combine(pas, pbs):
    """Host-side combination of the per-core partial vectors."""
    PA = np.stack([np.asarray(p).reshape(-1).astype(np.float64)
                   for p in pas])  # [8, NACC]
    PB = np.stack([np.asarray(p).reshape(-1).astype(np.float64)
                   for p in pbs])  # [8, NROW]

    focal = GAMMA_SCALE * PB[:, R_W].sum() / (B * HW)
    contrast = 0.5 * PA[:, K_CONTRAST].sum() / BF

    circ_total = 0.0
    for c in range(NCORES):
        for b in range(BP):
            area = PB[c, R_AREA + b]
            ex = PA[c, K_EX + b] + PA[c, K_BND + b]
            ey = PA[c, K_EY + 2 * b] + PA[c, K_EY + 2 * b + 1]
            per = ex + ey
            if area > 0 and per > 0:
                circv = 4.0 * np.pi * area / max(per, 1e-12) ** 2
                circ_total += (circv - 1.0) ** 2
    circ = 0.1 * circ_total / B

    S = PB[:, R_S:R_S + 3].sum(axis=0)
    I = PB[:, R_I:R_I + 3].sum(axis=0)
    cons_total = 0.0
    for k, (i, j) in enumerate(((0, 1), (0, 2), (1, 2))):
        union = S[i] + S[j] - I[k]
        iou = I[k] / (union + 1e-6)
        cons_total += max(0.6 - iou, 0.0)
    consensus = 0.3 * cons_total / 3.0

    return np.float32(focal + contrast + circ + consensus)


_CACHED_NC = None


def _get_nc():
    global _CACHED_NC
    if _CACHED_NC is None:
        _CACHED_NC = _build_nc()
    return _CACHED_NC


def kernel(logits, target, features, masks, method_preds):
    logits = np.asarray(logits, dtype=np.float32)
    target = np.asarray(target, dtype=np.int32)
    features = np.asarray(features, dtype=np.float32)
    masks = np.asarray(masks, dtype=np.float32)
    method_preds = np.asarray(method_preds, dtype=np.float32)

    in_maps = _host_inputs(logits, target, features, masks, method_preds)
    res = run_bass_kernel_spmd(_get_nc(), in_maps, list(range(NCORES)))
    pas = [res.results[c]["pa"] for c in range(NCORES)]
    pbs = [res.results[c]["pb"] for c in range(NCORES)]
    return _combine(pas, pbs)
